# revision 19
# baseline (speedup 1.0000x reference)
"""Trainium2 Bass kernel for nn_NeuralEmbeddingLayer.

Reference computation (B=32, T=512, N=1024):
  patches  = patchify(spikes)                       [B, 1024, 512]
  x1       = patches @ W_embed.T + b_embed          [B, 1024, 1024]
  x        = concat([cls, x1], axis=1)              [B, 1025, 1024]
  y        = gelu(x) * 32 @ W_proj.T + b_proj       [B, 1025, 1024]
  out      = y + pos_table                          (+ mask, stamp aux outputs)

Sharding: data-parallel over batch, 4 batches per core on 8 cores.
Weights replicated. The CLS row (row 0 of every batch) is input-independent
(one 1024-vector through the MLP) and is computed on host; the device
computes the 1024 patch rows per batch.

Per-core device pipeline (matmuls in bf16, fp32 accumulation):
  DMA patchify load (fp32) -> VectorE cast to bf16 -> xbar DMA transpose
  (contraction dim onto partitions) -> matmul1 (x1^T = W_embT.T @ A^T) ->
  ScalarE gelu(+b_embed per-partition bias, bf16 out) -> matmul2
  (y = g^T.T @ (32*W_proj).T) -> VectorE (+ pos_table + b_proj, fp32) ->
  DMA out.
"""

import math

import numpy as np
import ml_dtypes

import concourse.bass as bass
import concourse.mybir as mybir
import concourse.tile as tile
from concourse import bacc
from concourse.bass_utils import run_bass_kernel_spmd

# Problem constants
B, T, N = 32, 512, 1024
FT, FS = 16, 32
NCH = FT * FS            # 512  patch channels (contraction dim of mm1)
DIN = NCH * 2            # 1024
H = 1024
NCLS = 1
NP = (T // FT) * (N // FS)   # 1024 patches
SCALE = float(H) ** 0.5      # 32.0

NCORES = 8
BL = B // NCORES         # 4 batches per core

F32 = mybir.dt.float32
BF16 = mybir.dt.bfloat16


def build_nc(act_func=None):
    """Build the per-core Bass program (identical on all 8 cores).

    act_func: mybir.ActivationFunctionType override (default Gelu). CoreSim
    does not implement Gelu, so sim checks pass Tanh and compare against a
    numpy model using tanh.
    """
    if act_func is None:
        act_func = mybir.ActivationFunctionType.Gelu
    nc = bacc.Bacc(
        "TRN2",
        target_bir_lowering=False,
        debug=False,
        enable_asserts=False,
        num_devices=NCORES,
    )

    spikes = nc.dram_tensor("spikes", [BL, T, N], F32, kind="ExternalInput")
    w_embT = nc.dram_tensor("w_embT", [NCH, DIN], BF16, kind="ExternalInput")
    w_projT = nc.dram_tensor("w_projT", [DIN, H], BF16, kind="ExternalInput")
    pos2 = nc.dram_tensor("pos2", [NP, H], F32, kind="ExternalInput")
    b_emb = nc.dram_tensor("b_emb", [128, DIN // 128], F32, kind="ExternalInput")
    # one output tensor per g-block: Tile orders writes to the same DRAM
    # tensor, so a single tensor serializes all 32 stores end-to-end.
    outs = [
        nc.dram_tensor(f"out{g}", [BL, 128, H], F32, kind="ExternalOutput")
        for g in range(8)
    ]

    # DRAM iteration view for the patchified load.
    # t = (g*4+l)*16 + ft, n = pn*32 + fs; A rows (patches) = (g, l, pn),
    # patch channels c = (ft, fs). SBUF A tile: partition=(l,pn), free=(ft,fs).
    # DMA hardware limits each access pattern to 3 dims -> one DMA per (g, ft).
    spikes_r = spikes.ap().rearrange(
        "b (g l ft) (pn fs) -> b g l pn ft fs",
        g=8, l=4, ft=FT, pn=32, fs=FS,
    )

    with tile.TileContext(nc) as tc:
        with (
            tc.tile_pool(name="const", bufs=1) as cpool,
            tc.tile_pool(name="abuf", bufs=16) as apool,
            tc.tile_pool(name="abf", bufs=16) as abfpool,
            tc.tile_pool(name="atbuf", bufs=2) as atpool,
            tc.tile_pool(name="gtbuf", bufs=2) as gtpool,
            tc.tile_pool(name="outbuf", bufs=6) as opool,
            tc.tile_pool(name="tp_ps", bufs=4, space="PSUM") as tp_ps,
            tc.tile_pool(name="mm1_ps", bufs=2, space="PSUM") as mm1_ps,
            tc.tile_pool(name="mm2_ps", bufs=2, space="PSUM") as mm2_ps,
        ):
            # Resident constants
            w_embT_s = cpool.tile([128, 4, DIN], BF16)    # [c%128, cc, d]
            w_projT_s = cpool.tile([128, 8, H], BF16)     # [d%128, dc, h]
            pos2_s = cpool.tile([128, 8, H], F32)         # [p%128, pg, h]
            b_emb_s = cpool.tile([128, 8], F32)           # [d%128, dc]
            ident = cpool.tile([128, 128], BF16)
            from concourse.masks import make_identity
            make_identity(nc, ident[:])

            nc.scalar.dma_start(
                w_embT_s[:], w_embT.ap().rearrange("(cc p) d -> p cc d", p=128)
            )
            first = True
            b0_casts = []
            for b in range(BL):
                # ---- load (patchify gather) + cast to bf16 ----
                abf_g = []
                for g in range(8):
                    ab = apool.tile([128, NCH], F32, tag="abuf")
                    # one 64KB DMA per l: 32 contiguous partitions, free =
                    # (ft, fs); the 4 l-DMAs land on disjoint port quadrants
                    # and drain concurrently. Loads live on the sync ring
                    # (which carries nothing else); for the first batch the
                    # scalar ring (idle until the first gelu) takes half the
                    # loads to cut the pipeline-fill latency.
                    ld_eng = nc.scalar if (b == 0 and g % 2 == 1) else nc.sync
                    for l in range(4):
                        ld_eng.dma_start(
                            ab[l * 32:(l + 1) * 32, :], spikes_r[b, g, l]
                        )
                    abf = abfpool.tile([128, NCH], BF16, tag="abf")
                    cst = nc.vector.tensor_copy(abf[:], ab[:])
                    if b == 0:
                        b0_casts.append(cst)
                    abf_g.append(abf)

                if first:
                    # Bulky preloads go on the GPSIMD (SWDGE) ring AND are
                    # held back behind batch-0's casts: with no dependency
                    # they would issue at t=0 and their descriptors would
                    # saturate the DMA engines exactly when the first batch
                    # of spike loads needs them.
                    i_wp = nc.gpsimd.dma_start(
                        w_projT_s[:],
                        w_projT.ap().rearrange("(dc p) h -> p dc h", p=128),
                    )
                    tile.add_dep_helper(
                        i_wp.ins, b0_casts[4].ins, reason="delay w_projT preload"
                    )
                    i_pos = nc.gpsimd.dma_start(
                        pos2_s[:], pos2.ap().rearrange("(pg p) h -> p pg h", p=128)
                    )
                    tile.add_dep_helper(
                        i_pos.ins, b0_casts[6].ins, reason="delay pos2 preload"
                    )
                    nc.gpsimd.dma_start(b_emb_s[:], b_emb.ap())
                    first = False

                # ---- transpose A -> A^T on the PE (c onto partitions) ----
                # A^T split into per-(cc, mh) tiles so mm1 waits only on the
                # four transposes it actually reads, not the whole batch.
                at = {}
                for mh in range(2):
                    for cc in range(4):
                        at[(cc, mh)] = atpool.tile(
                            [128, 512], BF16, tag=f"at{cc}{mh}", name=f"at{cc}{mh}"
                        )
                for g in range(8):
                    mh, gl = g // 4, g % 4
                    for cc in range(4):
                        tp = tp_ps.tile([128, 128], BF16)
                        nc.tensor.transpose(
                            tp[:], abf_g[g][:, cc * 128:(cc + 1) * 128], ident[:]
                        )
                        nc.vector.tensor_copy(
                            at[(cc, mh)][:, gl * 128:(gl + 1) * 128], tp[:]
                        )

                # ---- mm1 + gelu: g^T[d, m] = gelu(W_embT.T @ A^T + b) ----
                # one PSUM bank (512 fp32) per matmul -> split m into halves
                gt = {}
                for mh in range(2):
                    for dc in range(8):
                        gt[(dc, mh)] = gtpool.tile(
                            [128, 512], BF16, tag=f"gt{dc}{mh}", name=f"gt{dc}{mh}"
                        )
                for mh in range(2):
                    for dc in range(8):
                        x1t = mm1_ps.tile([128, 512], F32)
                        for cc in range(4):
                            nc.tensor.matmul(
                                x1t[:],
                                w_embT_s[:, cc, dc * 128:(dc + 1) * 128],
                                at[(cc, mh)][:],
                                start=(cc == 0),
                                stop=(cc == 3),
                            )
                        nc.scalar.activation(
                            gt[(dc, mh)][:],
                            x1t[:],
                            act_func,
                            bias=b_emb_s[:, dc:dc + 1],
                        )

                # ---- mm2 + pos add + store ----
                for g in range(8):
                    out_t = opool.tile([128, H], F32)
                    for hc in range(2):
                        y = mm2_ps.tile([128, 512], F32)
                        for dc in range(8):
                            nc.tensor.matmul(
                                y[:],
                                gt[(dc, g // 4)][:, (g % 4) * 128:(g % 4 + 1) * 128],
                                w_projT_s[:, dc, hc * 512:(hc + 1) * 512],
                                start=(dc == 0),
                                stop=(dc == 7),
                            )
                        nc.vector.tensor_add(
                            out_t[:, hc * 512:(hc + 1) * 512],
                            y[:],
                            pos2_s[:, g, hc * 512:(hc + 1) * 512],
                        )
                    nc.scalar.dma_start(outs[g].ap()[b], out_t[:])

    nc.compile()
    return nc


def _host_prep(W_embed, b_embed, cls_tokens, W_proj, b_proj, pos_table):
    """Precompute device weight layouts + the (input-independent) CLS row."""
    w_embT = np.ascontiguousarray(W_embed.T).astype(ml_dtypes.bfloat16)       # [c, d]
    w_projT = np.ascontiguousarray((W_proj * SCALE).T).astype(ml_dtypes.bfloat16)  # [d, h]
    pos2 = (pos_table[NCLS:] + b_proj[None, :]).astype(np.float32)            # [NP, H]
    b_emb = np.ascontiguousarray(b_embed.reshape(8, 128).T).astype(np.float32)

    # CLS row: gelu(cls)*SCALE @ W_proj.T + b_proj + pos_table[0]
    v = cls_tokens.reshape(-1)[:DIN].astype(np.float64)
    erf = np.vectorize(math.erf)
    gelu_v = v * 0.5 * (1.0 + erf(v / math.sqrt(2.0)))
    cls_row = (gelu_v * SCALE) @ W_proj.astype(np.float64).T + b_proj + pos_table[0]
    cls_row = cls_row.astype(np.float32)
    return w_embT, w_projT, pos2, b_emb, cls_row


_NC_CACHE = {}


def kernel(spikes, W_embed, b_embed, cls_tokens, W_proj, b_proj, pos_table):
    spikes = np.asarray(spikes, dtype=np.float32)
    W_embed = np.asarray(W_embed, dtype=np.float32)
    b_embed = np.asarray(b_embed, dtype=np.float32)
    cls_tokens = np.asarray(cls_tokens, dtype=np.float32)
    W_proj = np.asarray(W_proj, dtype=np.float32)
    b_proj = np.asarray(b_proj, dtype=np.float32)
    pos_table = np.asarray(pos_table, dtype=np.float32)

    w_embT, w_projT, pos2, b_emb, cls_row = _host_prep(
        W_embed, b_embed, cls_tokens, W_proj, b_proj, pos_table
    )

    if "nc" not in _NC_CACHE:
        _NC_CACHE["nc"] = build_nc()
    nc = _NC_CACHE["nc"]

    in_maps = []
    for c in range(NCORES):
        in_maps.append({
            "spikes": np.ascontiguousarray(spikes[c * BL:(c + 1) * BL]),
            "w_embT": w_embT,
            "w_projT": w_projT,
            "pos2": pos2,
            "b_emb": b_emb,
        })

    res = run_bass_kernel_spmd(nc, in_maps, list(range(NCORES)))

    x = np.empty((B, NP + NCLS, H), dtype=np.float32)
    x[:, 0, :] = cls_row[None, :]
    for c in range(NCORES):
        for g in range(8):
            x[c * BL:(c + 1) * BL, NCLS + g * 128:NCLS + (g + 1) * 128, :] = (
                res.results[c][f"out{g}"]
            )

    mask = np.ones((B, NP + NCLS), dtype=np.int32)
    stamp = np.broadcast_to(
        np.arange(NP + NCLS, dtype=np.int32)[None, :], (B, NP + NCLS)
    ).copy()
    return x, mask, stamp


# revision 20
# speedup vs baseline: 1.1445x; 1.1445x over previous
"""Trainium2 Bass kernel for nn_NeuralEmbeddingLayer.

Reference computation (B=32, T=512, N=1024):
  patches  = patchify(spikes)                       [B, 1024, 512]
  x1       = patches @ W_embed.T + b_embed          [B, 1024, 1024]
  x        = concat([cls, x1], axis=1)              [B, 1025, 1024]
  y        = gelu(x) * 32 @ W_proj.T + b_proj       [B, 1025, 1024]
  out      = y + pos_table                          (+ mask, stamp aux outputs)

Sharding: data-parallel over batch, 4 batches per core on 8 cores.
Weights replicated. The CLS row (row 0 of every batch) is input-independent
(one 1024-vector through the MLP) and is computed on host; the device
computes the 1024 patch rows per batch.

Per-core device pipeline (matmuls in bf16, fp32 accumulation):
  DMA patchify load (fp32) -> VectorE cast to bf16 -> xbar DMA transpose
  (contraction dim onto partitions) -> matmul1 (x1^T = W_embT.T @ A^T) ->
  ScalarE gelu(+b_embed per-partition bias, bf16 out) -> matmul2
  (y = g^T.T @ (32*W_proj).T) -> VectorE (+ pos_table + b_proj, fp32) ->
  DMA out.
"""

import math

import numpy as np
import ml_dtypes

import concourse.bass as bass
import concourse.mybir as mybir
import concourse.tile as tile
from concourse import bacc
from concourse.bass_utils import run_bass_kernel_spmd

# Problem constants
B, T, N = 32, 512, 1024
FT, FS = 16, 32
NCH = FT * FS            # 512  patch channels (contraction dim of mm1)
DIN = NCH * 2            # 1024
H = 1024
NCLS = 1
NP = (T // FT) * (N // FS)   # 1024 patches
SCALE = float(H) ** 0.5      # 32.0

NCORES = 8
BL = B // NCORES         # 4 batches per core

F32 = mybir.dt.float32
BF16 = mybir.dt.bfloat16


def build_nc(act_func=None):
    """Build the per-core Bass program (identical on all 8 cores).

    act_func: mybir.ActivationFunctionType override (default Gelu). CoreSim
    does not implement Gelu, so sim checks pass Tanh and compare against a
    numpy model using tanh.
    """
    if act_func is None:
        act_func = mybir.ActivationFunctionType.Gelu
    nc = bacc.Bacc(
        "TRN2",
        target_bir_lowering=False,
        debug=False,
        enable_asserts=False,
        num_devices=NCORES,
    )

    spikes = nc.dram_tensor("spikes", [BL, T, N], F32, kind="ExternalInput")
    w_embT = nc.dram_tensor("w_embT", [NCH, DIN], BF16, kind="ExternalInput")
    w_projT = nc.dram_tensor("w_projT", [DIN, H], BF16, kind="ExternalInput")
    pos2 = nc.dram_tensor("pos2", [NP, H], F32, kind="ExternalInput")
    b_emb = nc.dram_tensor("b_emb", [128, DIN // 128], F32, kind="ExternalInput")
    # one output tensor per g-block: Tile orders writes to the same DRAM
    # tensor, so a single tensor serializes all 32 stores end-to-end.
    outs = [
        nc.dram_tensor(f"out{g}", [BL, 128, H], F32, kind="ExternalOutput")
        for g in range(8)
    ]

    # DRAM iteration view for the patchified load.
    # t = (g*4+l)*16 + ft, n = pn*32 + fs; A rows (patches) = (g, l, pn),
    # patch channels c = (ft, fs). SBUF A tile: partition=(l,pn), free=(ft,fs).
    # DMA hardware limits each access pattern to 3 dims -> one DMA per (g, ft).
    spikes_r = spikes.ap().rearrange(
        "b (g l ft) (pn fs) -> b g l pn ft fs",
        g=8, l=4, ft=FT, pn=32, fs=FS,
    )

    with tile.TileContext(nc) as tc:
        with (
            tc.tile_pool(name="const", bufs=1) as cpool,
            tc.tile_pool(name="abuf", bufs=16) as apool,
            tc.tile_pool(name="abf", bufs=16) as abfpool,
            tc.tile_pool(name="atbuf", bufs=2) as atpool,
            tc.tile_pool(name="gtbuf", bufs=2) as gtpool,
            tc.tile_pool(name="outbuf", bufs=6) as opool,
            tc.tile_pool(name="tp_ps", bufs=4, space="PSUM") as tp_ps,
            tc.tile_pool(name="mm1_ps", bufs=2, space="PSUM") as mm1_ps,
            tc.tile_pool(name="mm2_ps", bufs=2, space="PSUM") as mm2_ps,
        ):
            # Resident constants
            w_embT_s = cpool.tile([128, 4, DIN], BF16)    # [c%128, cc, d]
            w_projT_s = cpool.tile([128, 8, H], BF16)     # [d%128, dc, h]
            pos2_s = cpool.tile([128, 8, H], F32)         # [p%128, pg, h]
            b_emb_s = cpool.tile([128, 8], F32)           # [d%128, dc]
            ident = cpool.tile([128, 128], BF16)
            from concourse.masks import make_identity
            make_identity(nc, ident[:])

            nc.scalar.dma_start(
                w_embT_s[:], w_embT.ap().rearrange("(cc p) d -> p cc d", p=128)
            )
            first = True
            b0_casts = []
            for b in range(BL):
                # ---- load (patchify gather) + cast to bf16 ----
                abf_g = []
                for g in range(8):
                    ab = apool.tile([128, NCH], F32, tag="abuf")
                    # one 64KB DMA per l: 32 contiguous partitions, free =
                    # (ft, fs); the 4 l-DMAs land on disjoint port quadrants
                    # and drain concurrently. Loads live on the sync ring
                    # (which carries nothing else); for the first batch the
                    # scalar ring (idle until the first gelu) takes half the
                    # loads to cut the pipeline-fill latency.
                    ld_eng = nc.scalar if (b == 0 and g % 2 == 1) else nc.sync
                    for l in range(4):
                        ld_eng.dma_start(
                            ab[l * 32:(l + 1) * 32, :], spikes_r[b, g, l]
                        )
                    abf = abfpool.tile([128, NCH], BF16, tag="abf")
                    cst = nc.vector.tensor_copy(abf[:], ab[:])
                    if b == 0:
                        b0_casts.append(cst)
                    abf_g.append(abf)

                if first:
                    # Bulky preloads go on the GPSIMD (SWDGE) ring AND are
                    # held back behind batch-0's casts: with no dependency
                    # they would issue at t=0 and their descriptors would
                    # saturate the DMA engines exactly when the first batch
                    # of spike loads needs them.
                    nc.gpsimd.dma_start(b_emb_s[:], b_emb.ap())
                    i_wp = nc.gpsimd.dma_start(
                        w_projT_s[:],
                        w_projT.ap().rearrange("(dc p) h -> p dc h", p=128),
                    )
                    tile.add_dep_helper(
                        i_wp.ins, b0_casts[2].ins, reason="delay w_projT preload"
                    )
                    i_pos = nc.gpsimd.dma_start(
                        pos2_s[:], pos2.ap().rearrange("(pg p) h -> p pg h", p=128)
                    )
                    tile.add_dep_helper(
                        i_pos.ins, b0_casts[5].ins, reason="delay pos2 preload"
                    )
                    first = False

                # ---- transpose A -> A^T on the PE (c onto partitions) ----
                # A^T split into per-(cc, mh) tiles so mm1 waits only on the
                # four transposes it actually reads, not the whole batch.
                at = {}
                for mh in range(2):
                    for cc in range(4):
                        at[(cc, mh)] = atpool.tile(
                            [128, 512], BF16, tag=f"at{cc}{mh}", name=f"at{cc}{mh}"
                        )
                for g in range(8):
                    mh, gl = g // 4, g % 4
                    for cc in range(4):
                        tp = tp_ps.tile([128, 128], BF16)
                        nc.tensor.transpose(
                            tp[:], abf_g[g][:, cc * 128:(cc + 1) * 128], ident[:]
                        )
                        nc.vector.tensor_copy(
                            at[(cc, mh)][:, gl * 128:(gl + 1) * 128], tp[:]
                        )

                # ---- mm1 + gelu: g^T[d, m] = gelu(W_embT.T @ A^T + b) ----
                # one PSUM bank (512 fp32) per matmul -> split m into halves
                gt = {}
                for mh in range(2):
                    for dc in range(8):
                        gt[(dc, mh)] = gtpool.tile(
                            [128, 512], BF16, tag=f"gt{dc}{mh}", name=f"gt{dc}{mh}"
                        )
                for mh in range(2):
                    for dc in range(8):
                        x1t = mm1_ps.tile([128, 512], F32)
                        for cc in range(4):
                            nc.tensor.matmul(
                                x1t[:],
                                w_embT_s[:, cc, dc * 128:(dc + 1) * 128],
                                at[(cc, mh)][:],
                                start=(cc == 0),
                                stop=(cc == 3),
                            )
                        nc.scalar.activation(
                            gt[(dc, mh)][:],
                            x1t[:],
                            act_func,
                            bias=b_emb_s[:, dc:dc + 1],
                        )

                # ---- mm2 + pos add + store ----
                for g in range(8):
                    out_t = opool.tile([128, H], F32)
                    for hc in range(2):
                        y = mm2_ps.tile([128, 512], F32)
                        for dc in range(8):
                            nc.tensor.matmul(
                                y[:],
                                gt[(dc, g // 4)][:, (g % 4) * 128:(g % 4 + 1) * 128],
                                w_projT_s[:, dc, hc * 512:(hc + 1) * 512],
                                start=(dc == 0),
                                stop=(dc == 7),
                            )
                        nc.vector.tensor_add(
                            out_t[:, hc * 512:(hc + 1) * 512],
                            y[:],
                            pos2_s[:, g, hc * 512:(hc + 1) * 512],
                        )
                    nc.scalar.dma_start(outs[g].ap()[b], out_t[:])

    nc.compile()
    return nc


def _host_prep(W_embed, b_embed, cls_tokens, W_proj, b_proj, pos_table):
    """Precompute device weight layouts + the (input-independent) CLS row."""
    w_embT = np.ascontiguousarray(W_embed.T).astype(ml_dtypes.bfloat16)       # [c, d]
    w_projT = np.ascontiguousarray((W_proj * SCALE).T).astype(ml_dtypes.bfloat16)  # [d, h]
    pos2 = (pos_table[NCLS:] + b_proj[None, :]).astype(np.float32)            # [NP, H]
    b_emb = np.ascontiguousarray(b_embed.reshape(8, 128).T).astype(np.float32)

    # CLS row: gelu(cls)*SCALE @ W_proj.T + b_proj + pos_table[0]
    v = cls_tokens.reshape(-1)[:DIN].astype(np.float64)
    erf = np.vectorize(math.erf)
    gelu_v = v * 0.5 * (1.0 + erf(v / math.sqrt(2.0)))
    cls_row = (gelu_v * SCALE) @ W_proj.astype(np.float64).T + b_proj + pos_table[0]
    cls_row = cls_row.astype(np.float32)
    return w_embT, w_projT, pos2, b_emb, cls_row


_NC_CACHE = {}


def kernel(spikes, W_embed, b_embed, cls_tokens, W_proj, b_proj, pos_table):
    spikes = np.asarray(spikes, dtype=np.float32)
    W_embed = np.asarray(W_embed, dtype=np.float32)
    b_embed = np.asarray(b_embed, dtype=np.float32)
    cls_tokens = np.asarray(cls_tokens, dtype=np.float32)
    W_proj = np.asarray(W_proj, dtype=np.float32)
    b_proj = np.asarray(b_proj, dtype=np.float32)
    pos_table = np.asarray(pos_table, dtype=np.float32)

    w_embT, w_projT, pos2, b_emb, cls_row = _host_prep(
        W_embed, b_embed, cls_tokens, W_proj, b_proj, pos_table
    )

    if "nc" not in _NC_CACHE:
        _NC_CACHE["nc"] = build_nc()
    nc = _NC_CACHE["nc"]

    in_maps = []
    for c in range(NCORES):
        in_maps.append({
            "spikes": np.ascontiguousarray(spikes[c * BL:(c + 1) * BL]),
            "w_embT": w_embT,
            "w_projT": w_projT,
            "pos2": pos2,
            "b_emb": b_emb,
        })

    res = run_bass_kernel_spmd(nc, in_maps, list(range(NCORES)))

    x = np.empty((B, NP + NCLS, H), dtype=np.float32)
    x[:, 0, :] = cls_row[None, :]
    for c in range(NCORES):
        for g in range(8):
            x[c * BL:(c + 1) * BL, NCLS + g * 128:NCLS + (g + 1) * 128, :] = (
                res.results[c][f"out{g}"]
            )

    mask = np.ones((B, NP + NCLS), dtype=np.int32)
    stamp = np.broadcast_to(
        np.arange(NP + NCLS, dtype=np.int32)[None, :], (B, NP + NCLS)
    ).copy()
    return x, mask, stamp
